# revision 25
# baseline (speedup 1.0000x reference)
"""Trainium2 Bass kernel for nn_BondMatrixMessage (GNN bond-matrix message passing).

Primary path (M2, per-node P-route; one NeuronCore per batch, programs are
compiled per core because edge-group widths depend on connectivity):

    P[n,k,i] = sum_j W[k,i,j] h[n,j]        (dense N x (K*D) GEMM on PE)
    m[e,:]   = P[src_e]^T b_e               (per-8-node-group matmuls)
    out[t]   = sum_{e->t} m[e]              (position-scatter + S-matmuls)

vs. the v1 fallback (kept below) which expands bw[e] = b_e @ W per EDGE
(E x D*D elements through PSUM), the P-route only pushes N x K*D elements
through PSUM and needs no per-edge DVE multiply of the bw expansion.

  - class(n)=n&1 picks the partition half, rank=n>>1, slot=rank&3, group=n>>3.
    P2[(64c+k), r, i] (SBUF bf16, 128KB/partition) is filled by 256 F=512
    GEMMs (class 0 -> PSUM partitions 0-63, class 1 -> 64-127 via the
    out-AP partition base) + 128 strided evacs alternating ACT(2/3)/DVE(1/3).
  - Node matmuls: stationary = P2[:, 4G:4G+4, :] (8 consecutive nodes as
    2 classes x 4 slots), moving = bond2 (class-zero-padded bond columns,
    host-sorted by src>>3). Output (128=(slot,i), cols) has 3 garbage slots
    per column; a host mask (DVE mult from PSUM) kills them, then 4 tiny
    matmuls per 512-tile with the masked product as STATIONARY and a slot-sum
    selector as moving give edge-major m chunks directly (no transposes).
  - A(q)/B interleave: B-tiles of quarter q-1 are emitted between A(q)'s
    i-iterations so PE always has GEMM work while B's cross-engine chain runs.
  - Scatter: m rows go to DRAM md2 at their tgt-sorted position via
    dma_scatter_add (tpos is a bijection -> no in-call duplicates; md2 cols
    0-31 are pre-zeroed since scatter-add is RMW). Aggregation = 128-edge
    chunks of md2 as matmul STATIONARY against a host 0/1 S-table (fp8)
    accumulating into outT (32, 4096) PSUM: chunk target-spans are contiguous
    and sorted, so each chunk is 1 fresh-column matmul (start=True) plus at
    most 1 overlap column (start=False). PE transposes outT -> out rows.
"""
import sys

sys.path.insert(0, "/opt/trn_rl_repo")

import numpy as np

from concourse import bacc, bass, mybir, tile, bass_utils

# problem constants (hardcoded per spec)
B = 8
N = 4096
E = 16384
D = 32          # atom dim
KB = 64         # bond dim
TIL = 512       # edges per pipeline tile
NT = E // TIL   # 32 tiles
CH = 8          # (j,i) chunks per tile
NBLK = 16       # sorted-mod blocks (requires max in-degree <= NBLK)
TPB = E // NBLK  # tokens per block = 1024
NCOPY = 4       # DRAM accumulator copies (block b -> copy b % NCOPY)
GATH = 1        # tiles per dma_gather call (>1 overflows the SWDGE ring on HW)
NPAIR = CH // 2  # chunk pairs per tile (each pair = one 2-bank PSUM tile)
DVE_DIRECT_PAIRS = 1  # pairs whose multiply reads PSUM directly on DVE
MCOPY_ACT = 1   # m_all evac copies per tile done on ACT (rest on DVE)
F32 = mybir.dt.float32
BF16 = mybir.dt.bfloat16
I16 = mybir.dt.int16

_PROGRAM_CACHE = {}

# tunables: SBUF/PSUM pool depths and op-fusion knobs
CFG = dict(
    pair=True,            # fuse chunk pairs into 2-bank PSUM tiles
    bw_bufs=3,            # PSUM bufs for bw tiles (x2 banks if pair)
    mt_bufs=1,            # PSUM bufs for the mT accumulator
    tp_bufs=1,            # PSUM bufs for transpose outputs (0 = share mt pool)
    pt_bufs=8,
    bwsb_bufs=4,
    bt_bufs=8,
    dve_direct_pairs=1,
    dve_direct_chunks=2,
    mcopy_act=1,
    warm_gathers=False,
    nq=1,
)


def _build_program(cfg=None):
    cfg = {**CFG, **(cfg or {})}
    nc = bacc.Bacc("TRN2", target_bir_lowering=False, debug=False, num_devices=B)

    atab_din = nc.dram_tensor("atab", (N, 128), BF16, kind="ExternalInput")
    bondT_d = nc.dram_tensor("bondT", (KB, E), BF16, kind="ExternalInput")
    w2_d = nc.dram_tensor("w2", (KB, CH * 128), BF16, kind="ExternalInput")
    sel_d = nc.dram_tensor("sel", (128, CH * D), BF16, kind="ExternalInput")
    ident_d = nc.dram_tensor("ident", (D, D), F32, kind="ExternalInput")
    srcw_d = nc.dram_tensor("srcw", (128, E // 16), I16, kind="ExternalInput")
    tgtw_d = nc.dram_tensor("tgtw", (128, E // 16), I16, kind="ExternalInput")
    out_d = nc.dram_tensor("out", (N, D), F32, kind="ExternalOutput")

    with tile.TileContext(nc) as tc:
        with tc.tile_pool(name="const", bufs=1) as cp, \
             tc.tile_pool(name="work", bufs=cfg["pt_bufs"]) as wp, \
             tc.tile_pool(name="bwsb", bufs=cfg["bwsb_bufs"]) as bp, \
             tc.tile_pool(name="btp", bufs=cfg["bt_bufs"]) as btp, \
             tc.tile_pool(name="srp", bufs=2) as sp, \
             tc.tile_pool(name="mtev", bufs=2) as mp, \
             tc.tile_pool(name="redu", bufs=2) as rp, \
             tc.tile_pool(name="bwps", bufs=cfg["bw_bufs"], space="PSUM") as bwp, \
             tc.tile_pool(name="mtps", bufs=cfg["mt_bufs"], space="PSUM") as mtp, \
             tc.tile_pool(name="tpps", bufs=max(cfg["tp_bufs"], 1), space="PSUM") as tpp, \
             tc.tile_pool(name="dram", bufs=1, space="DRAM") as dp:
            tp_pool = tpp if cfg["tp_bufs"] > 0 else mtp
            tp_tag = "tp" if cfg["tp_bufs"] > 0 else "mt"

            # ---- one-time setup (srcw first: tile 0's gather needs it) ----
            srcw_sb = cp.tile([128, E // 16], I16)
            nc.sync.dma_start(srcw_sb[:], srcw_d.ap())
            w2_sb = cp.tile([KB, CH * 128], BF16)
            nc.sync.dma_start(w2_sb[:], w2_d.ap())
            sel_sb = cp.tile([128, CH * D], BF16)
            nc.sync.dma_start(sel_sb[:], sel_d.ap())
            ident_sb = cp.tile([D, D], F32)
            nc.sync.dma_start(ident_sb[:], ident_d.ap())
            tgtw_sb = cp.tile([128, E // 16], I16)
            nc.scalar.dma_start(tgtw_sb[:], tgtw_d.ap())

            # gather table T[n, 4j+r] = atom[n, j] (host-prepped bf16 input)
            atab_d = atab_din

            # edge-major messages, token-wrapped: token q at [q%128, q//128, 0:32]
            # (memset emitted after tile 0's gather: Pool SEQ issues in order,
            # and the two big memsets would otherwise delay the first gather)
            m_all = cp.tile([128, E // 128, 64], F32)
            zero_sb = cp.tile([128, (N // 128) * 64], F32)
            copies = [dp.tile([N, 64], F32, name=f"copy{c}") for c in range(NCOPY)]

            def _deferred_setup():
                nc.gpsimd.memset(m_all[:], 0.0)
                nc.gpsimd.memset(zero_sb[:], 0.0)
                for c in range(NCOPY):
                    nc.scalar.dma_start(
                        copies[c][:].rearrange("(p g) j -> p (g j)", p=128),
                        zero_sb[:],
                    )

            # ---- main pipeline ----
            for t in range(NT):
                esl = slice(t * TIL, (t + 1) * TIL)

                bt_sb = btp.tile([KB, TIL], BF16, tag="bt")
                bt_eng = nc.scalar if cfg.get("bt_on_act") else nc.sync
                bt_eng.dma_start(bt_sb[:], bondT_d.ap()[:, esl])

                # optionally: first two gathers cover 1 tile each (fast start)
                nwarm = 2 if cfg.get("warm_gathers", True) else 0
                if t < nwarm or (t - nwarm) % GATH == 0:
                    ng = 1 if t < nwarm else min(GATH, NT - t)
                    srep = sp.tile([128, 1, GATH * TIL], BF16, tag="srep")
                    nidx = ng * TIL
                    nc.gpsimd.dma_gather(
                        out_ap=srep[:, :, :nidx],
                        in_ap=atab_d.ap(),
                        idxs_ap=srcw_sb[:, t * (TIL // 16):(t + ng) * (TIL // 16)],
                        num_idxs=nidx,
                        num_idxs_reg=nidx,
                        elem_size=128,
                        transpose=True,
                    )
                    srep_base = t
                ssl = slice((t - srep_base) * TIL, (t - srep_base + 1) * TIL)

                if t == 0:
                    _deferred_setup()

                mt_ps = mtp.tile([D, TIL], F32, tag="mt")
                if cfg["pair"]:
                    # srep broadcast over a chunk pair: [2 (step 0), TIL (step 1)]
                    srep_pair = srep[:, 0:1, ssl].to_broadcast([128, 2, TIL])
                    for pr in range(NPAIR):
                        bw_ps = bwp.tile([128, 2, TIL], F32, tag="bw")
                        for h in range(2):
                            c = 2 * pr + h
                            nc.tensor.matmul(
                                out=bw_ps[:, h, :],
                                lhsT=w2_sb[:, c * 128:(c + 1) * 128],
                                rhs=bt_sb[:],
                                start=True, stop=True,
                            )
                        pt_sb = wp.tile([128, 2, TIL], BF16, tag="pt")
                        if pr < cfg["dve_direct_pairs"]:
                            nc.vector.tensor_tensor(
                                out=pt_sb[:], in0=bw_ps[:], in1=srep_pair,
                                op=mybir.AluOpType.mult,
                            )
                        else:
                            bw_sb = bp.tile([128, 2, TIL], BF16, tag="bwsb")
                            nc.scalar.copy(bw_sb[:], bw_ps[:])
                            nc.vector.tensor_tensor(
                                out=pt_sb[:], in0=bw_sb[:], in1=srep_pair,
                                op=mybir.AluOpType.mult,
                            )
                        for h in range(2):
                            c = 2 * pr + h
                            nc.tensor.matmul(
                                out=mt_ps[:],
                                lhsT=sel_sb[:, c * D:(c + 1) * D],
                                rhs=pt_sb[:, h, :],
                                start=(c == 0), stop=(c == CH - 1),
                            )
                else:
                    ndir = cfg.get("dve_direct_chunks", 2 * cfg["dve_direct_pairs"])
                    for c in range(CH):
                        bw_ps = bwp.tile([128, TIL], F32, tag="bw")
                        nc.tensor.matmul(
                            out=bw_ps[:],
                            lhsT=w2_sb[:, c * 128:(c + 1) * 128],
                            rhs=bt_sb[:],
                            start=True, stop=True,
                        )
                        pt_sb = wp.tile([128, TIL], BF16, tag="pt")
                        if c < ndir:
                            nc.vector.tensor_tensor(
                                out=pt_sb[:], in0=bw_ps[:], in1=srep[:, 0, ssl],
                                op=mybir.AluOpType.mult,
                            )
                        else:
                            bw_sb = bp.tile([128, TIL], BF16, tag="bwsb")
                            nc.scalar.copy(bw_sb[:], bw_ps[:])
                            eng = (nc.gpsimd if c >= CH - cfg.get("gp_chunks", 0)
                                   else nc.vector)
                            eng.tensor_tensor(
                                out=pt_sb[:], in0=bw_sb[:], in1=srep[:, 0, ssl],
                                op=mybir.AluOpType.mult,
                            )
                        nc.tensor.matmul(
                            out=mt_ps[:],
                            lhsT=sel_sb[:, c * D:(c + 1) * D],
                            rhs=pt_sb[:],
                            start=(c == 0), stop=(c == CH - 1),
                        )

                mt_sb = mp.tile([D, TIL], F32, tag="mtsb")
                if cfg.get("mt_evac_dve"):
                    nc.vector.tensor_copy(mt_sb[:], mt_ps[:])
                else:
                    nc.scalar.copy(mt_sb[:], mt_ps[:])

                for q in range(TIL // 128):
                    tp_ps = tp_pool.tile([128, D], F32, tag=tp_tag, name="tp_ps")
                    nc.tensor.transpose(
                        tp_ps[:], mt_sb[:, q * 128:(q + 1) * 128], ident_sb[:]
                    )
                    slot = t * (TIL // 128) + q
                    if q < cfg["mcopy_act"]:
                        nc.scalar.copy(m_all[:, slot, 0:D], tp_ps[:])
                    else:
                        nc.vector.tensor_copy(m_all[:, slot, 0:D], tp_ps[:])

                # one block (= 2 tiles = 1024 tokens) completed -> scatter it
                if t % 2 == 1:
                    blk = t // 2
                    nc.gpsimd.dma_scatter_add(
                        out_ap=copies[blk // (NBLK // NCOPY)][:],
                        in_ap=m_all[:, blk * (TPB // 128):(blk + 1) * (TPB // 128), :],
                        idxs_ap=tgtw_sb[:, blk * (TPB // 16):(blk + 1) * (TPB // 16)],
                        num_idxs=TPB,
                        num_idxs_reg=TPB,
                        elem_size=64,
                    )

            # ---- final reduce of the copies, split over node ranges ----
            # copy tensors and out use p-major node layout: row n = 32*p + g
            NQ = cfg.get("nq", 4)
            GQ = (N // 128) // NQ  # node-groups per range
            for q in range(NQ):
                acc = rp.tile([128, GQ * D], F32, tag="acc")
                for c in range(NCOPY):
                    csl = rp.tile([128, GQ, D], F32, tag="csl")
                    nc.sync.dma_start(
                        csl[:],
                        copies[c][:].rearrange("(p g) j -> p g j", p=128)
                        [:, q * GQ:(q + 1) * GQ, 0:D],
                    )
                    if c == 0:
                        nc.vector.tensor_copy(
                            acc[:], csl[:].rearrange("p g j -> p (g j)")
                        )
                    else:
                        nc.vector.tensor_tensor(
                            out=acc[:], in0=acc[:],
                            in1=csl[:].rearrange("p g j -> p (g j)"),
                            op=mybir.AluOpType.add,
                        )
                nc.sync.dma_start(
                    out_d.ap().rearrange("(p g) j -> p g j", p=128)
                    [:, q * GQ:(q + 1) * GQ, :].rearrange("p g j -> p (g j)"),
                    acc[:],
                )

    nc.compile()
    return nc


def _host_prep(atom_state, bond_state, bond_transform, connectivity):
    """Build per-core input maps. Pure layout / index-metadata / dtype prep."""
    import ml_dtypes

    W = np.asarray(bond_transform, dtype=np.float32)  # (KB, D*D)

    # W2[k, c*128 + p] = W[k, (4c + p%4)*D + p//4]   (i = 4c + p%4, j = p//4)
    p = np.arange(128)
    cc = np.arange(CH)
    i_idx = 4 * cc[:, None] + (p % 4)[None, :]   # (CH, 128)
    j_idx = np.broadcast_to((p // 4)[None, :], (CH, 128))
    w2 = W[:, (i_idx * D + j_idx).reshape(-1)].astype(ml_dtypes.bfloat16)

    # selectors S_c[p, m] = [4c + p%4 == m]
    sel = np.zeros((128, CH * D), dtype=np.float32)
    for c in range(CH):
        sel[p, c * D + 4 * c + (p % 4)] = 1.0
    sel_bf = sel.astype(ml_dtypes.bfloat16)

    ident = np.eye(D, dtype=np.float32)

    in_maps = []
    for b in range(B):
        src = np.asarray(connectivity[b, :, 0], dtype=np.int64)
        tgt = np.asarray(connectivity[b, :, 1], dtype=np.int64)
        order = np.argsort(tgt, kind="stable")
        deg = np.bincount(tgt, minlength=N).max()
        if deg > NBLK:
            raise ValueError(f"max in-degree {deg} exceeds {NBLK}")
        # processing order: blocks by sorted_pos % NBLK
        proc = np.concatenate([order[c::NBLK] for c in range(NBLK)])
        srcp = src[proc].astype(np.int16)
        tgtp = tgt[proc].astype(np.int16)

        bondT = np.ascontiguousarray(
            np.asarray(bond_state[b], dtype=np.float32).T[:, proc]
        ).astype(ml_dtypes.bfloat16)  # (KB, E)

        # wrapped idx tables: idxs[p, s] = vals[16*s + p%16], tiled to 128 partitions
        def wrap16(vals):
            w = vals.reshape(-1, 16).T  # (16, E//16)
            return np.ascontiguousarray(np.tile(w, (8, 1)), dtype=np.int16)

        # gather table T[n, 4j+r] = atom[n, j]
        atab = np.repeat(
            np.asarray(atom_state[b], dtype=np.float32), 4, axis=1
        ).astype(ml_dtypes.bfloat16)

        in_maps.append({
            "atab": np.ascontiguousarray(atab),
            "bondT": bondT,
            "w2": w2,
            "sel": sel_bf,
            "ident": ident,
            "srcw": wrap16(srcp),
            "tgtw": wrap16(tgtp),
        })
    return in_maps


# ======================= M2: per-node P-route =======================
#
# P[n,k,i] = sum_j W[k,i,j] h[n,j]  (dense per-node GEMM, N x (K*D))
# m[e,:]  = P[src_e]^T b_e          (per-8-node-group matmuls, slot-packed)
# out[t]  = sum_{e->t} m[e]         (position-scatter to tgt-order + S-matmuls)
#
# class(n)=n&1 (partition half), rank=n>>1, slot=rank&3, group=n>>3.
# P2[(64c+k), r, i] = P[2r+c, k, i]; group G stationary = P2[:, 4G:4G+4, :].
# bond2 col e: rows 64c..64c+64 = b_e for c=class(src_e), else 0.
# mask row 32s+i = [s == slot(src_e)] kills the 3 garbage slots after the
# group matmul; sel (128,32) sums slots -> mt (32, TIL).

NSLOT = 4
NGRP = N // 8      # 512 groups of 8 consecutive nodes
SCHK = 128         # edges per S-aggregation chunk (tgt-sorted)


def _host_prep2(atom_state, bond_state, bond_transform, connectivity):
    import ml_dtypes

    BF = ml_dtypes.bfloat16
    F8 = ml_dtypes.float8_e4m3
    W3 = np.asarray(bond_transform, np.float32).reshape(KB, D, D)
    # wtall[j, i*64+k] = W[k, i, j]
    wtall = np.ascontiguousarray(
        W3.transpose(2, 1, 0).reshape(D, D * KB)).astype(BF)
    sel = np.zeros((128, D), np.float32)
    for s in range(NSLOT):
        sel[32 * s + np.arange(D), np.arange(D)] = 1.0
    sel = sel.astype(BF)
    ident = np.eye(D, dtype=np.float32)

    def wrap16(vals):
        w = vals.reshape(-1, 16).T
        return np.ascontiguousarray(np.tile(w, (8, 1)), dtype=np.int16)

    cores = []
    for b in range(B):
        src = np.asarray(connectivity[b, :, 0], np.int64)
        tgt = np.asarray(connectivity[b, :, 1], np.int64)
        order = np.argsort(src >> 3, kind="stable")
        srcp, tgtp = src[order], tgt[order]
        grp = srcp >> 3
        gstart = np.searchsorted(grp, np.arange(NGRP), side="left")
        gend = np.searchsorted(grp, np.arange(NGRP), side="right")

        # htc[j, c*2048 + r] = atom[2r+c, j]
        h = np.asarray(atom_state[b], np.float32)
        htc = np.empty((D, N), np.float32)
        htc[:, :N // 2] = h[0::2].T
        htc[:, N // 2:] = h[1::2].T
        htc = np.ascontiguousarray(htc).astype(BF)

        bsort = np.asarray(bond_state[b], np.float32)[order]
        cls = (srcp & 1)
        bond2 = np.zeros((128, E), np.float32)
        for c in (0, 1):
            mask_c = cls == c
            bond2[64 * c:64 * c + KB, mask_c] = bsort[mask_c].T
        bond2 = bond2.astype(BF)

        slot = (srcp >> 1) & 3
        mask = np.zeros((128, E), np.float32)
        for s in range(NSLOT):
            ms = slot == s
            mask[32 * s:32 * s + D, ms] = 1.0
        mask = mask.astype(BF)

        # scatter: position in tgt-sorted order
        tsort = np.argsort(tgtp, kind="stable")
        tpos = np.empty(E, np.int64)
        tpos[tsort] = np.arange(E)
        sidx = wrap16(tpos.astype(np.int16))

        # S-table: per 128-edge chunk of tgt-sorted order
        ts_tgt = tgtp[tsort]
        spans, los = [], []
        scols = [0]
        for c in range(E // SCHK):
            lo = int(ts_tgt[SCHK * c])
            hi = int(ts_tgt[SCHK * c + SCHK - 1])
            los.append(lo)
            spans.append(hi - lo + 1)
            scols.append(scols[-1] + spans[-1])
        stab = np.zeros((SCHK, scols[-1]), np.float32)
        for c in range(E // SCHK):
            seg = ts_tgt[SCHK * c:SCHK * (c + 1)] - los[c]
            stab[np.arange(SCHK), scols[c] + seg] = 1.0
        stab = np.ascontiguousarray(stab).astype(F8)

        cores.append(dict(
            in_map={
                "wtall": wtall, "htc": htc, "bond2": bond2, "mask": mask,
                "sel": sel, "ident": ident, "sidx": sidx, "stab": stab,
            },
            gstart=gstart, gend=gend,
            spans=spans, los=los, scols=scols, scol_total=scols[-1],
        ))
    return cores


def _build_program2(meta):
    gstart, gend = meta["gstart"], meta["gend"]
    spans, los, scols = meta["spans"], meta["los"], meta["scols"]
    SCOL = meta["scol_total"]
    NQ4 = NGRP // 4     # groups per quarter
    F8D = mybir.dt.float8e4

    nc = bacc.Bacc("TRN2", target_bir_lowering=False, debug=False,
                   num_devices=1)
    wtall_d = nc.dram_tensor("wtall", (D, D * KB), BF16, kind="ExternalInput")
    htc_d = nc.dram_tensor("htc", (D, N), BF16, kind="ExternalInput")
    bond2_d = nc.dram_tensor("bond2", (128, E), BF16, kind="ExternalInput")
    mask_d = nc.dram_tensor("mask", (128, E), BF16, kind="ExternalInput")
    sel_d = nc.dram_tensor("sel", (128, D), BF16, kind="ExternalInput")
    ident_d = nc.dram_tensor("ident", (D, D), F32, kind="ExternalInput")
    sidx_d = nc.dram_tensor("sidx", (128, E // 16), I16, kind="ExternalInput")
    stab_d = nc.dram_tensor("stab", (SCHK, SCOL), F8D, kind="ExternalInput")
    out_d = nc.dram_tensor("out", (N, D), F32, kind="ExternalOutput")

    # per-tile group segments: (G, col0, col1)
    tile_segs = [[] for _ in range(NT)]
    for G in range(NGRP):
        c0, c1 = int(gstart[G]), int(gend[G])
        while c0 < c1:
            t = c0 // TIL
            ce = min(c1, (t + 1) * TIL)
            tile_segs[t].append((G, c0, ce))
            c0 = ce
    # first tile whose groups are all < quarter boundary gets processed
    # after A(q); tile t needs quarter ceil of its max group
    tile_quarter = [min(3, max(G for G, _, _ in segs) // NQ4) if segs else 0
                    for segs in tile_segs]

    with tile.TileContext(nc) as tc:
        with tc.tile_pool(name="const", bufs=1) as cp, \
             tc.tile_pool(name="dram", bufs=1, space="DRAM") as dp:
            # ---- persistent tiles ----
            sel_sb = cp.tile([128, D], BF16)
            nc.sync.dma_start(sel_sb[:], sel_d.ap())
            ident_sb = cp.tile([D, D], F32)
            nc.sync.dma_start(ident_sb[:], ident_d.ap())
            sidx_sb = cp.tile([128, E // 16], I16)
            nc.scalar.dma_start(sidx_sb[:], sidx_d.ap())
            stab_sb = cp.tile([SCHK, SCOL], F8D)
            nc.scalar.dma_start(stab_sb[:], stab_d.ap())
            p2_sb = cp.tile([128, N // 2, D], BF16)      # 128 KB/partition
            md2 = dp.tile([E, 128], BF16, name="md2")
            zero_sb = cp.tile([128, 8, D], BF16)
            nc.gpsimd.memset(zero_sb[:], 0.0)
            md2z = md2[:].rearrange("(g p) j -> p g j", p=128)
            for s in range(16):
                nc.scalar.dma_start(md2z[:, s * 8:(s + 1) * 8, 0:D],
                                    zero_sb[:])


            with tc.tile_pool(name="wa", bufs=1) as wa, \
                 tc.tile_pool(name="btp", bufs=6) as btp, \
                 tc.tile_pool(name="mkp", bufs=6) as mkp, \
                 tc.tile_pool(name="ptp", bufs=6) as ptp, \
                 tc.tile_pool(name="mal", bufs=1) as malp, \
                 tc.tile_pool(name="ppq", bufs=3, space="PSUM") as ppq, \
                 tc.tile_pool(name="bw2", bufs=3, space="PSUM") as bw2, \
                 tc.tile_pool(name="tpp", bufs=2, space="PSUM") as tpp:
                wtall_sb = wa.tile([D, D * KB], BF16)
                nc.sync.dma_start(wtall_sb[:], wtall_d.ap())
                htc_sb = wa.tile([D, N], BF16)
                nc.sync.dma_start(htc_sb[:], htc_d.ap())
                m_all = malp.tile([128, E // 128, 128], BF16)
                memset_done = [False] * 4

                done_tiles = 0

                def emit_tile(t):
                    esl = slice(t * TIL, (t + 1) * TIL)
                    bt = btp.tile([128, TIL], BF16, tag="bt")
                    nc.sync.dma_start(bt[:], bond2_d.ap()[:, esl])
                    mk = mkp.tile([128, TIL], BF16, tag="mk")
                    nc.scalar.dma_start(mk[:], mask_d.ap()[:, esl])

                    pp2 = bw2.tile([128, TIL], F32, tag="pp2")
                    for (G, c0, c1) in tile_segs[t]:
                        if c1 <= c0:
                            continue
                        nc.tensor.matmul(
                            out=pp2[:, c0 - t * TIL:c1 - t * TIL],
                            lhsT=p2_sb[:, 4 * G:4 * G + 4, :],
                            rhs=bt[:, c0 - t * TIL:c1 - t * TIL],
                            start=True, stop=True,
                        )
                    pt = ptp.tile([128, TIL], BF16, tag="pt")
                    nc.vector.tensor_tensor(
                        out=pt[:], in0=pp2[:], in1=mk[:],
                        op=mybir.AluOpType.mult,
                    )
                    # m chunks edge-major: stationary = pt chunk (128, 128),
                    # moving = sel (128, 32): out[e, i] = sum_s pt[(s,i), e]
                    mch = tpp.tile([128, TIL // 128, D], F32, tag="tp")
                    for q in range(TIL // 128):
                        nc.tensor.matmul(
                            out=mch[:, q, :],
                            lhsT=pt[:, q * 128:(q + 1) * 128],
                            rhs=sel_sb[:],
                            start=True, stop=True,
                        )
                    if t % 2 == 0:
                        nc.scalar.copy(
                            m_all[:, t * 4:(t + 1) * 4, 0:D], mch[:])
                    else:
                        nc.vector.tensor_copy(
                            m_all[:, t * 4:(t + 1) * 4, 0:D], mch[:])
                    mq = t // 8
                    if not memset_done[mq]:
                        nc.gpsimd.memset(
                            m_all[:, mq * 32:(mq + 1) * 32, D:], 0.0)
                        memset_done[mq] = True
                    if t % 2 == 1:
                        blk = t // 2
                        nc.gpsimd.dma_scatter_add(
                            out_ap=md2[:],
                            in_ap=m_all[:, blk * 8:(blk + 1) * 8, :],
                            idxs_ap=sidx_sb[:, blk * 64:(blk + 1) * 64],
                            num_idxs=1024,
                            num_idxs_reg=1024,
                            elem_size=128,
                        )

                # ---- interleave: A(q) i-iters with B tiles of q-1 ----
                for q4 in range(4):
                    rsl = slice(q4 * (N // 8), (q4 + 1) * (N // 8))  # ranks
                    r0 = q4 * (N // 8)
                    ready = [t for t in range(done_tiles, NT)
                             if tile_quarter[t] <= q4 - 1]
                    per = max(1, (D + len(ready)) // max(len(ready), 1)) \
                        if ready else D + 1
                    for i in range(D):
                        pp = ppq.tile([128, N // 8], F32, tag="pp")
                        for c in (0, 1):
                            nc.tensor.matmul(
                                out=pp[64 * c:64 * c + KB, :],
                                lhsT=wtall_sb[:, i * KB:(i + 1) * KB],
                                rhs=htc_sb[:, c * (N // 2) + r0:
                                           c * (N // 2) + r0 + N // 8],
                                start=True, stop=True,
                            )
                        if i % 3 != 2:
                            nc.scalar.copy(p2_sb[:, rsl, i], pp[:])
                        else:
                            nc.vector.tensor_copy(p2_sb[:, rsl, i], pp[:])
                        while (ready and done_tiles <= ready[0]
                               and done_tiles < NT
                               and tile_quarter[done_tiles] <= q4 - 1
                               and (i + 1) % per == 0):
                            emit_tile(done_tiles)
                            done_tiles += 1
                            ready.pop(0)
                    while (done_tiles < NT
                           and tile_quarter[done_tiles] <= q4 - 1):
                        emit_tile(done_tiles)
                        done_tiles += 1
                while done_tiles < NT:
                    emit_tile(done_tiles)
                    done_tiles += 1

            # ---- tail: gather-free aggregation via S matmuls ----
            with tc.tile_pool(name="m2p", bufs=3) as m2p, \
                 tc.tile_pool(name="osb", bufs=1) as osbp:
                m2_tiles = []
                md2r = md2[:].rearrange("(c p) j -> p c j", p=128)
                outT_sb = osbp.tile([D, N], F32)
                with tc.tile_pool(name="otp", bufs=1, space="PSUM") as otp:
                    outT = otp.tile([D, N], F32)
                    nc.vector.memset(outT[:], 0.0)
                    for tl in range(E // 1024):
                        m2 = m2p.tile([128, 8, D], BF16, tag="m2")
                        eng = nc.sync if tl % 2 == 0 else nc.scalar
                        eng.dma_start(
                            m2[:], md2r[:, tl * 8:(tl + 1) * 8, 0:D])
                        for cc in range(8):
                            c = tl * 8 + cc
                            lo, sp = los[c], spans[c]
                            prev_hi = los[c - 1] + spans[c - 1] - 1 if c else -1
                            nov = max(0, prev_hi - lo + 1)  # overlap cols
                            if nov:
                                nc.tensor.matmul(
                                    out=outT[:, lo:lo + nov],
                                    lhsT=m2[:, cc, :],
                                    rhs=stab_sb[:, scols[c]:scols[c] + nov],
                                    start=False, stop=True,
                                    skip_group_check=True,
                                )
                            if sp > nov:
                                nc.tensor.matmul(
                                    out=outT[:, lo + nov:lo + sp],
                                    lhsT=m2[:, cc, :],
                                    rhs=stab_sb[:, scols[c] + nov:
                                                scols[c] + sp],
                                    start=True, stop=True,
                                    skip_group_check=True,
                                )
                    nc.scalar.copy(outT_sb[:], outT[:])
                outsb = osbp.tile([128, N // 128, D], F32)
                with tc.tile_pool(name="tp2", bufs=2, space="PSUM") as tp2p:
                    for g4 in range(N // 512):
                        tp2 = tp2p.tile([128, 4, D], F32, tag="tp2")
                        for gg in range(4):
                            g = g4 * 4 + gg
                            nc.tensor.transpose(
                                tp2[:, gg, :],
                                outT_sb[:, g * 128:(g + 1) * 128],
                                ident_sb[:],
                            )
                        nc.vector.tensor_copy(
                            outsb[:, g4 * 4:(g4 + 1) * 4, :], tp2[:])
                nc.sync.dma_start(
                    out_d.ap().rearrange("(g p) j -> p g j", p=128),
                    outsb[:],
                )

    nc.compile()
    return nc


def kernel(atom_state, bond_state, bond_transform, connectivity):
    try:
        cores = _host_prep2(atom_state, bond_state, bond_transform,
                            connectivity)
        outs = []
        for b in range(B):
            key = ("m2", connectivity[b].tobytes())
            if key not in _PROGRAM_CACHE:
                _PROGRAM_CACHE[key] = _build_program2(cores[b])
            nc = _PROGRAM_CACHE[key]
            _PROGRAM_CACHE.setdefault("m2_ncs", {})[b] = nc
            res = bass_utils.run_bass_kernel_spmd(nc, [cores[b]["in_map"]], [0])
            outs.append(res.results[0]["out"])
        return np.stack(outs, axis=0).astype(np.float32)
    except Exception as ex:
        import traceback
        traceback.print_exc()
        print(f"M2 path failed ({ex}); falling back to v1")
        if "nc" not in _PROGRAM_CACHE:
            _PROGRAM_CACHE["nc"] = _build_program()
        nc = _PROGRAM_CACHE["nc"]
        in_maps = _host_prep(atom_state, bond_state, bond_transform,
                             connectivity)
        res = bass_utils.run_bass_kernel_spmd(nc, in_maps, list(range(B)))
        out = np.stack([res.results[b]["out"] for b in range(B)], axis=0)
        return out.astype(np.float32)

